# revision 72
# baseline (speedup 1.0000x reference)
"""Trainium2 Bass kernel for nn_Attention_42288247996512 (sparse causal cross-attention).

reference:
  q = x @ Wq.T; k = cross @ Wk.T; v = x @ Wv.T
  logits = q @ k.T  (causal mask; padding mask m_q*m_k + eye > 0)
  out = softmax(logits / sqrt(128)) @ v

Sharding: 8 cores = 4 batches x 2 query-strips. Each strip is 8 query blocks
(128 rows) chosen so both strips have identical causal-chunk structure
(SPMD: one program, per-core data). Host pre-transposes inputs (avoids
on-chip fp32 transposes), pre-scales Wq by 1/sqrt(128), builds additive
mask tiles, and does the final denominator divide + scatter.

On-chip per core: kT/qT/v projections (float32r matmuls), then per
block-pair: logits -> +mask (DVE) -> exp (ACT, per-partition q-mask bias,
accum_out denominator) -> PE transpose -> AV matmul -> store out.T.
"""
import math
import os
import threading

import ml_dtypes
import numpy as np

B, S, D, DA = 4, 2048, 1024, 128
P = 128
NCORES = 8
BIG = 32768.0  # power of two: exactly representable in bf16
NBLK = S // P  # 16 key blocks / query blocks per batch
NQ = 1024      # query rows per core strip

# strips: pairs of adjacent blocks, same chunk-count multiset on both strips
STRIPS = [
    [0, 1, 14, 15, 6, 7, 8, 9],
    [2, 3, 12, 13, 4, 5, 10, 11],
]
PAIR_C = [1, 4, 2, 3]  # 512-wide key chunks per pair (same for both strips)

_BUILD_LOCK = threading.Lock()
_CACHE: dict = {}


def _build():
    from contextlib import ExitStack

    import concourse.bass as bass
    import concourse.mybir as mybir
    import concourse.tile as tile
    from concourse import bacc
    from concourse.masks import make_identity

    dt = mybir.dt
    f32 = dt.float32
    f32r = dt.float32r
    AF = mybir.ActivationFunctionType
    ALU = mybir.AluOpType

    nc = bacc.Bacc("TRN2", target_bir_lowering=False, debug=False)

    bf16 = dt.bfloat16
    xT = nc.dram_tensor("xT", [D, S], f32r, kind="ExternalInput").ap()
    cT = nc.dram_tensor("cT", [D, S], f32r, kind="ExternalInput").ap()
    xqT = nc.dram_tensor("xqT", [D, NQ], f32r, kind="ExternalInput").ap()
    wqT = nc.dram_tensor("wqT", [D, DA], f32r, kind="ExternalInput").ap()
    wkT = nc.dram_tensor("wkT", [D, DA], f32r, kind="ExternalInput").ap()
    wvT = nc.dram_tensor("wvT", [D, D], f32r, kind="ExternalInput").ap()
    # additive masks in bf16 (values are sums of +-2^15/2^16: exact in bf16)
    kmb = nc.dram_tensor("kmb", [P, 1536], bf16, kind="ExternalInput").ap()
    qmn = nc.dram_tensor("qmn", [P, 8], f32, kind="ExternalInput").ap()
    dmask = nc.dram_tensor("dmask", [8, P, 512], bf16, kind="ExternalInput").ap()

    outT = nc.dram_tensor("outT", [D, NQ], f32, kind="ExternalOutput").ap()
    den = nc.dram_tensor("den", [P, 8], f32, kind="ExternalOutput").ap()

    KC = D // P  # 8 contraction chunks for projections

    with tile.TileContext(nc) as tc, ExitStack() as ctx:
        const = ctx.enter_context(tc.tile_pool(name="const", bufs=1))
        persist = ctx.enter_context(tc.tile_pool(name="persist", bufs=1))
        stream = ctx.enter_context(tc.tile_pool(name="stream", bufs=2))

        # ---- constants / weights / masks ----
        ident = const.tile([P, P], f32, name="ident")
        make_identity(nc, ident)

        # All input DMAs go on the single SP HWDGE queue: one InstDMACopy
        # spreads over all 16 SDMA slots (full ~358GB/s), and the strict
        # FIFO gives exact control of delivery order = consumption order.
        # Tiles are declared here; their loads are emitted at the point in
        # the phase schedule where the FIFO should deliver them.
        wvT_r = wvT.rearrange("(kc p) m -> kc p m", p=P)
        wq_sb = const.tile([P, KC, DA], f32r, name="wq_sb")
        wk_sb = const.tile([P, KC, DA], f32r, name="wk_sb")
        kmb_sb = const.tile([P, 1536], bf16, name="kmb_sb")
        qmn_sb = const.tile([P, 8], f32, name="qmn_sb")
        dm_sb = const.tile([P, 8, 512], bf16, name="dm_sb")

        kT_sb = persist.tile([P, S], f32r, name="kT_sb")
        qT_sb = persist.tile([P, NQ], f32r, name="qT_sb")
        v_sb = persist.tile([P, NBLK, D], f32r, name="v_sb")
        den_sb = persist.tile([P, 8], f32, name="den_sb")

        # ---- fused schedule ----
        # R1: v-eighths with qT / kT-chalf0 chunklets interleaved per kc
        #     (spreads their DMA bursts); psv 4 + pj 2 PSUM banks.
        # R2: attention pairs 0,2 interleaved with kT-chalf1 (whose psum
        #     rides the psT tag); psl 2 + psT 2 + psav 4 = 8 banks.
        # R3: attention pairs 3,1 (DMA long done; pure PE).
        with ExitStack() as phase_ctx:
            pj = phase_ctx.enter_context(
                tc.tile_pool(name="pj", bufs=1, space="PSUM"))
            wvpool = phase_ctx.enter_context(
                tc.tile_pool(name="wvpool", bufs=1))
            xpool = phase_ctx.enter_context(
                tc.tile_pool(name="xpool", bufs=3))
            wv_sb = wvpool.tile([P, KC, D], f32r, name="wv_sb")
            eTs_all = {pr: [] for pr in range(4)}
            daccs_all = {pr: [[], []] for pr in range(4)}
            proj_ps = {}

            def qT_chunklet(qkc):
                if qkc == 0:
                    nc.sync.dma_start(
                        wq_sb[:], wqT.rearrange("(kc p) m -> p kc m", p=P))
                    proj_ps["q"] = [pj.tile([P, 512], f32, tag=f"pj{n}",
                                            name=f"ps_q{n}") for n in range(2)]
                xq = stream.tile([P, NQ], f32r, tag="xq", name=f"xq{qkc}",
                                 bufs=3)
                nc.sync.dma_start(xq[:], xqT[qkc * P:(qkc + 1) * P, :])
                for n in range(2):
                    nc.tensor.matmul(
                        proj_ps["q"][n][:],
                        lhsT=wq_sb[:, qkc, :],
                        rhs=xq[:, n * 512:(n + 1) * 512],
                        start=(qkc == 0), stop=(qkc == KC - 1),
                    )
                if qkc == KC - 1:
                    for n in range(2):
                        nc.any.tensor_copy(
                            qT_sb[:, n * 512:(n + 1) * 512],
                            proj_ps["q"][n][:])

            def kT_chunklet(chalf, kkc, ps_pool=None, ps_tag="pj"):
                if kkc == 0:
                    if chalf == 0:
                        nc.sync.dma_start(
                            wk_sb[:],
                            wkT.rearrange("(kc p) m -> p kc m", p=P))
                    pool_ = ps_pool if ps_pool is not None else pj
                    proj_ps[chalf] = [
                        pool_.tile([P, 512], f32, tag=f"{ps_tag}{n}",
                                   name=f"ps_k{chalf}_{n}") for n in range(2)]
                ps_k = proj_ps[chalf]
                ct = stream.tile([P, NQ], f32r, tag="ct",
                                 name=f"ct{chalf}_{kkc}", bufs=3)
                nc.sync.dma_start(
                    ct[:], cT[kkc * P:(kkc + 1) * P,
                              chalf * NQ:(chalf + 1) * NQ])
                for n in range(2):
                    nc.tensor.matmul(
                        ps_k[n][:],
                        lhsT=wk_sb[:, kkc, :],
                        rhs=ct[:, n * 512:(n + 1) * 512],
                        start=(kkc == 0), stop=(kkc == KC - 1),
                    )
                if kkc == KC - 1:
                    for n in range(2):
                        nc.any.tensor_copy(
                            kT_sb[:, chalf * NQ + n * 512:
                                  chalf * NQ + (n + 1) * 512], ps_k[n][:])

            psv = phase_ctx.enter_context(
                tc.tile_pool(name="psv", bufs=1, space="PSUM"))
            if True:

                xT_r = xT.rearrange("(kc p) s -> p kc s", p=P)

                def v_eighth(se, dma_hook=None, mm_hook=None,
                             defer_copies=False, split_xt=False):
                    # One strided DMA loads all 8 kc chunks of this eighth
                    # (each dma_start costs ~650ns of SP sequencer issue
                    # time, so fewer/bigger DMAs beat per-chunk loads).
                    # split_xt: ve0 loads per-kc so the PE can consume each
                    # (wv[kc], xt[kc]) pair as it lands at startup.
                    if split_xt:
                        xta = xpool.tile([P, KC, 256], f32r, tag="xts",
                                         name=f"xts{se}", bufs=2)
                        for kc in range(KC):
                            if dma_hook is not None:
                                dma_hook(kc)
                            nc.sync.dma_start(
                                xta[:, kc, :],
                                xT[kc * P:(kc + 1) * P,
                                   se * 256:(se + 1) * 256])
                    else:
                        xta = xpool.tile([P, KC, 256], f32r, tag="xt",
                                         name=f"xta{se}")
                        # kc-quarter loads: the kc-ascending matmuls
                        # start as soon as the first quarter lands
                        for h in range(4):
                            nc.sync.dma_start(
                                xta[:, 2 * h:2 * h + 2, :],
                                xT_r[:, 2 * h:2 * h + 2,
                                     se * 256:(se + 1) * 256])
                        if dma_hook is not None:
                            for kc in range(KC):
                                dma_hook(kc)
                    pss = [psv.tile([P, 512], f32, tag=f"psv{i}",
                                    name=f"psv{se}_{i}") for i in range(4)]
                    for kc in range(KC):
                        for n in range(2):
                            for sb in range(2):
                                nc.tensor.matmul(
                                    pss[sb * 2 + n][:],
                                    lhsT=xta[:, kc, sb * P:(sb + 1) * P],
                                    rhs=wv_sb[:, kc, n * 512:(n + 1) * 512],
                                    start=(kc == 0), stop=(kc == KC - 1),
                                )
                        if mm_hook is not None:
                            mm_hook(kc)
                    def flush():
                        for sb in range(2):
                            for n in range(2):
                                nc.any.tensor_copy(
                                    v_sb[:, se * 2 + sb,
                                         n * 512:(n + 1) * 512],
                                    pss[sb * 2 + n][:])
                    if defer_copies:
                        return flush
                    flush()
                    return None

                v_eighth(0, dma_hook=lambda kc: nc.sync.dma_start(
                    wv_sb[:, kc, :], wvT_r[kc]), split_xt=True)
                # qT / kT-chalf0 chunklets spread 2-3 per eighth so each
                # hosting eighth stays DMA-surplus-positive.
                def qh(base):
                    return lambda kc: qT_chunklet(base + kc // 4) \
                        if kc % 4 == 3 else None

                def kh(lst):
                    return lambda kc: kT_chunklet(0, lst[kc // 3]) \
                        if kc % 3 == 1 and kc // 3 < len(lst) else None

                v_eighth(1, mm_hook=qh(0), split_xt=True)
                v_eighth(2, mm_hook=qh(2))
                v_eighth(3, mm_hook=qh(4))
                v_eighth(4, mm_hook=qh(6))
                v_eighth(5, mm_hook=kh([0, 1, 2]))
                v_eighth(6, mm_hook=kh([3, 4, 5]))
                # the SP FIFO tail is xt-slot gated, so these mask loads
                # slip into idle DMA windows without delaying ve7
                nc.sync.dma_start(qmn_sb[:], qmn[:])
                nc.sync.dma_start(dm_sb[:], dmask.rearrange("s p t -> p s t"))
                nc.sync.dma_start(kmb_sb[:], kmb[:])
                v_eighth(7, mm_hook=kh([6, 7]))

        # ---- attention (stage A: logits->exp->transpose; B: AV) ----
        if True:
            apool = ctx.enter_context(tc.tile_pool(name="apool", bufs=5))
            epool = ctx.enter_context(tc.tile_pool(name="epool", bufs=24))
            with tc.tile_pool(name="psl", bufs=2, space="PSUM") as psl_pool, \
                 tc.tile_pool(name="psT", bufs=2, space="PSUM") as psT_pool, \
                 tc.tile_pool(name="psav", bufs=2, space="PSUM") as psav_pool, \
                 tc.tile_pool(name="pjk", bufs=1, space="PSUM") as pjk_pool:

                def stage_a_chunk(pr, j, mid_hook=None):
                    c = PAIR_C[pr]
                    psTs = [psT_pool.tile([P, 256], f32, tag="psT",
                                          name=f"psT{pr}_{j}_{ks}",
                                          padded_shape=[P, 512])
                            for ks in range(4)]
                    es = []
                    for blk in range(2):
                        slot = pr * 2 + blk
                        psl = psl_pool.tile([P, 512], f32, tag="psl",
                                            name=f"psl{slot}_{j}")
                        nc.tensor.matmul(
                            psl[:],
                            lhsT=qT_sb[:, slot * P:(slot + 1) * P],
                            rhs=kT_sb[:, j * 512:(j + 1) * 512],
                            start=True, stop=True,
                        )
                        sbl = apool.tile([P, 512], f32, tag="sbl",
                                         name=f"sbl{slot}_{j}")
                        add_src = dm_sb[:, slot, :] if j == c - 1 \
                            else kmb_sb[:, j * 512:(j + 1) * 512]
                        nc.vector.tensor_tensor(
                            out=sbl[:], in0=psl[:], in1=add_src, op=ALU.add)
                        e = apool.tile([P, 512], f32, tag="e",
                                       name=f"e{slot}_{j}")
                        dac = apool.tile([P, 1], f32, tag="dac",
                                         name=f"dac{slot}_{j}", bufs=10)
                        nc.scalar.activation(
                            e[:], sbl[:], AF.Exp,
                            bias=qmn_sb[:, slot:slot + 1], scale=1.0,
                            accum_out=dac[:],
                        )
                        daccs_all[pr][blk].append(dac)
                        es.append(e)
                    # PE work emitted here hides the DVE-add + exp latency
                    # before the transposes need the exp outputs
                    if mid_hook is not None:
                        mid_hook()
                    for blk in range(2):
                        for ks in range(4):
                            nc.tensor.transpose(
                                psTs[ks][:, blk * P:(blk + 1) * P],
                                es[blk][:, ks * P:(ks + 1) * P],
                                ident[:],
                            )
                    for ks in range(4):
                        eT = epool.tile([P, 256], f32r, tag="eT",
                                        name=f"eT{pr}_{j}_{ks}")
                        nc.any.tensor_copy(eT[:], psTs[ks][:])
                        eTs_all[pr].append(eT)

                def stage_b(pr, use_pjk=False, den_dma=False):
                    c = PAIR_C[pr]
                    eTs = eTs_all[pr]
                    for blk in range(2):
                        slot = pr * 2 + blk
                        dl = daccs_all[pr][blk]
                        dst = den_sb[:, slot:slot + 1]
                        if c == 1:
                            nc.any.tensor_copy(dst, dl[0][:])
                        else:
                            nc.vector.tensor_tensor(
                                out=dst, in0=dl[0][:], in1=dl[1][:],
                                op=ALU.add)
                            for d in dl[2:]:
                                nc.vector.tensor_tensor(
                                    out=dst, in0=dst, in1=d[:], op=ALU.add)
                    if den_dma:
                        # all 8 den slots are final here; flush during the AV
                        nc.sync.dma_start(den[:], den_sb[:])
                    for dmc in range(8):
                        if use_pjk and dmc % 4 >= 2:
                            # kT-chalf1's pjk banks are dead by R3: reuse
                            # them as two extra AV slots (depth 4 pipeline)
                            psav = pjk_pool.tile(
                                [P, 512], f32, tag=f"pjk{dmc % 2}",
                                name=f"psav{pr}_{dmc}")[:, :256]
                        else:
                            psav = psav_pool.tile([P, 256], f32, tag="psav",
                                                  name=f"psav{pr}_{dmc}")
                        for kb in range(4 * c):
                            nc.tensor.matmul(
                                psav[:],
                                lhsT=v_sb[:, kb, dmc * P:(dmc + 1) * P],
                                rhs=eTs[kb][:],
                                start=(kb == 0), stop=(kb == 4 * c - 1),
                            )
                        osb = apool.tile([P, 256], f32, tag="osb",
                                         name=f"osb{pr}_{dmc}")
                        nc.any.tensor_copy(osb[:], psav[:])
                        nc.sync.dma_start(
                            outT[dmc * P:(dmc + 1) * P,
                                 pr * 256:(pr + 1) * 256], osb[:])

                # R2: pairs 0 and 2 interleaved with kT chalf1 (psum on psT tag)
                def kt1_first():
                    for kkc in range(4):
                        kT_chunklet(1, kkc, ps_pool=pjk_pool, ps_tag="pjk")

                def kt1_second():
                    for kkc in range(4, 8):
                        kT_chunklet(1, kkc, ps_pool=pjk_pool, ps_tag="pjk")

                stage_a_chunk(0, 0, mid_hook=kt1_first)
                stage_b(0)
                stage_a_chunk(2, 0, mid_hook=kt1_second)
                stage_a_chunk(2, 1)
                stage_b(2)
                # R3: pair 1 then 3, A3 interleaved before B1
                for j in range(4):
                    stage_a_chunk(1, j)
                stage_a_chunk(3, 0)
                stage_b(1, use_pjk=True)
                stage_a_chunk(3, 1)
                stage_a_chunk(3, 2)
                stage_b(3, use_pjk=True, den_dma=True)


    nc.compile()
    return nc


def _get_nc():
    with _BUILD_LOCK:
        if "nc" not in _CACHE:
            _CACHE["nc"] = _build()
        return _CACHE["nc"]


def kernel(x, cross, Wq, Wk, Wv, mask):
    from concourse import bass_utils

    nc = _get_nc()

    x = np.asarray(x, dtype=np.float32)
    cross = np.asarray(cross, dtype=np.float32)
    scale = 1.0 / math.sqrt(DA)
    wqT_h = np.ascontiguousarray((np.asarray(Wq, np.float32) * scale).T)
    wkT_h = np.ascontiguousarray(np.asarray(Wk, np.float32).T)
    wvT_h = np.ascontiguousarray(np.asarray(Wv, np.float32).T)
    mf = np.asarray(mask).astype(np.float32)  # [B, S]

    karange = np.arange(S)
    in_maps = []
    rows_per_core = []
    for core in range(NCORES):
        b, p = divmod(core, 2)
        blocks = STRIPS[p]
        rows = np.concatenate([np.arange(g * P, (g + 1) * P) for g in blocks])
        rows_per_core.append((b, rows))
        mb = mf[b]
        kneg = (-BIG * (1.0 - mb)).astype(np.float32)  # [S]
        kmb_h = np.ascontiguousarray(
            np.broadcast_to(kneg[:1536], (P, 1536))).astype(ml_dtypes.bfloat16)
        mq = mb[rows]  # [1024]
        qmn_h = np.ascontiguousarray(
            (-BIG * (1.0 - mq)).reshape(8, P).T)  # [128, 8]
        dm_h = np.empty((8, P, 512), np.float32)
        for s, g in enumerate(blocks):
            c = PAIR_C[s // 2]
            k0 = (c - 1) * 512
            kk = karange[k0:k0 + 512]
            qq = g * P + np.arange(P)
            mqs = mq[s * P:(s + 1) * P]
            t = np.broadcast_to(kneg[k0:k0 + 512], (P, 512)).copy()
            t += -BIG * (kk[None, :] > qq[:, None])
            t += (2.0 * BIG * (1.0 - mqs))[:, None] * (kk[None, :] == qq[:, None])
            dm_h[s] = t
        in_maps.append({
            "xT": np.ascontiguousarray(x[b].T),
            "cT": np.ascontiguousarray(cross[b].T),
            "xqT": np.ascontiguousarray(x[b][rows].T),
            "wqT": wqT_h,
            "wkT": wkT_h,
            "wvT": wvT_h,
            "kmb": kmb_h,
            "qmn": qmn_h,
            "dmask": dm_h.astype(ml_dtypes.bfloat16),
        })

    _CACHE["in_maps"] = in_maps
    res = bass_utils.run_bass_kernel_spmd(
        nc, in_maps, core_ids=list(range(NCORES)))

    out = np.empty((B, S, D), np.float32)
    for core in range(NCORES):
        b, rows = rows_per_core[core]
        r = res.results[core]
        o = r["outT"].T  # [1024 q, 1024 dm]
        denf = r["den"].T.reshape(-1)  # [1024] strip-ordered
        out[b, rows] = o / denf[:, None]
    return out


# revision 76
# speedup vs baseline: 1.0132x; 1.0132x over previous
"""Trainium2 Bass kernel for nn_Attention_42288247996512 (sparse causal cross-attention).

reference:
  q = x @ Wq.T; k = cross @ Wk.T; v = x @ Wv.T
  logits = q @ k.T  (causal mask; padding mask m_q*m_k + eye > 0)
  out = softmax(logits / sqrt(128)) @ v

Sharding: 8 cores = 4 batches x 2 query-strips. Each strip is 8 query blocks
(128 rows) chosen so both strips have identical causal-chunk structure
(SPMD: one program, per-core data). Host pre-transposes inputs (avoids
on-chip fp32 transposes), pre-scales Wq by 1/sqrt(128), builds additive
mask tiles, and does the final denominator divide + scatter.

On-chip per core: kT/qT/v projections (float32r matmuls), then per
block-pair: logits -> +mask (DVE) -> exp (ACT, per-partition q-mask bias,
accum_out denominator) -> PE transpose -> AV matmul -> store out.T.
"""
import math
import os
import threading

import ml_dtypes
import numpy as np

B, S, D, DA = 4, 2048, 1024, 128
P = 128
NCORES = 8
BIG = 32768.0  # power of two: exactly representable in bf16
NBLK = S // P  # 16 key blocks / query blocks per batch
NQ = 1024      # query rows per core strip

# strips: pairs of adjacent blocks, same chunk-count multiset on both strips
STRIPS = [
    [0, 1, 14, 15, 6, 7, 8, 9],
    [2, 3, 12, 13, 4, 5, 10, 11],
]
PAIR_C = [1, 4, 2, 3]  # 512-wide key chunks per pair (same for both strips)

_BUILD_LOCK = threading.Lock()
_CACHE: dict = {}


def _build():
    from contextlib import ExitStack

    import concourse.bass as bass
    import concourse.mybir as mybir
    import concourse.tile as tile
    from concourse import bacc
    from concourse.masks import make_identity

    dt = mybir.dt
    f32 = dt.float32
    f32r = dt.float32r
    AF = mybir.ActivationFunctionType
    ALU = mybir.AluOpType

    nc = bacc.Bacc("TRN2", target_bir_lowering=False, debug=False)

    bf16 = dt.bfloat16
    xT = nc.dram_tensor("xT", [D, S], f32r, kind="ExternalInput").ap()
    cT = nc.dram_tensor("cT", [D, S], f32r, kind="ExternalInput").ap()
    xqT = nc.dram_tensor("xqT", [D, NQ], f32r, kind="ExternalInput").ap()
    wqT = nc.dram_tensor("wqT", [D, DA], f32r, kind="ExternalInput").ap()
    wkT = nc.dram_tensor("wkT", [D, DA], f32r, kind="ExternalInput").ap()
    wvT = nc.dram_tensor("wvT", [D, D], f32r, kind="ExternalInput").ap()
    # additive masks in bf16 (values are sums of +-2^15/2^16: exact in bf16)
    kmb = nc.dram_tensor("kmb", [P, 1536], bf16, kind="ExternalInput").ap()
    qmn = nc.dram_tensor("qmn", [P, 8], f32, kind="ExternalInput").ap()
    dmask = nc.dram_tensor("dmask", [8, P, 512], bf16, kind="ExternalInput").ap()

    outT = nc.dram_tensor("outT", [D, NQ], f32, kind="ExternalOutput").ap()
    den = nc.dram_tensor("den", [P, 8], f32, kind="ExternalOutput").ap()

    KC = D // P  # 8 contraction chunks for projections

    with tile.TileContext(nc) as tc, ExitStack() as ctx:
        const = ctx.enter_context(tc.tile_pool(name="const", bufs=1))
        persist = ctx.enter_context(tc.tile_pool(name="persist", bufs=1))
        stream = ctx.enter_context(tc.tile_pool(name="stream", bufs=2))

        # ---- constants / weights / masks ----
        ident_f32 = const.tile([P, P], f32, name="ident_f32")
        make_identity(nc, ident_f32)
        ident = const.tile([P, P], f32r, name="ident")
        nc.vector.tensor_copy(ident[:], ident_f32[:])

        # All input DMAs go on the single SP HWDGE queue: one InstDMACopy
        # spreads over all 16 SDMA slots (full ~358GB/s), and the strict
        # FIFO gives exact control of delivery order = consumption order.
        # Tiles are declared here; their loads are emitted at the point in
        # the phase schedule where the FIFO should deliver them.
        wvT_r = wvT.rearrange("(kc p) m -> kc p m", p=P)
        wq_sb = const.tile([P, KC, DA], f32r, name="wq_sb")
        wk_sb = const.tile([P, KC, DA], f32r, name="wk_sb")
        kmb_sb = const.tile([P, 1536], bf16, name="kmb_sb")
        qmn_sb = const.tile([P, 8], f32, name="qmn_sb")
        dm_sb = const.tile([P, 8, 512], bf16, name="dm_sb")

        kT_sb = persist.tile([P, S], f32r, name="kT_sb")
        qT_sb = persist.tile([P, NQ], f32r, name="qT_sb")
        v_sb = persist.tile([P, NBLK, D], f32r, name="v_sb")
        den_sb = persist.tile([P, 8], f32, name="den_sb")

        # ---- fused schedule ----
        # R1: v-eighths with qT / kT-chalf0 chunklets interleaved per kc
        #     (spreads their DMA bursts); psv 4 + pj 2 PSUM banks.
        # R2: attention pairs 0,2 interleaved with kT-chalf1 (whose psum
        #     rides the psT tag); psl 2 + psT 2 + psav 4 = 8 banks.
        # R3: attention pairs 3,1 (DMA long done; pure PE).
        with ExitStack() as phase_ctx:
            pj = phase_ctx.enter_context(
                tc.tile_pool(name="pj", bufs=1, space="PSUM"))
            wvpool = phase_ctx.enter_context(
                tc.tile_pool(name="wvpool", bufs=1))
            xpool = phase_ctx.enter_context(
                tc.tile_pool(name="xpool", bufs=3))
            wv_sb = wvpool.tile([P, KC, D], f32r, name="wv_sb")
            eTs_all = {pr: [] for pr in range(4)}
            daccs_all = {pr: [[], []] for pr in range(4)}
            proj_ps = {}

            def qT_chunklet(qkc):
                if qkc == 0:
                    nc.sync.dma_start(
                        wq_sb[:], wqT.rearrange("(kc p) m -> p kc m", p=P))
                    proj_ps["q"] = [pj.tile([P, 512], f32, tag=f"pj{n}",
                                            name=f"ps_q{n}") for n in range(2)]
                xq = stream.tile([P, NQ], f32r, tag="xq", name=f"xq{qkc}",
                                 bufs=3)
                nc.sync.dma_start(xq[:], xqT[qkc * P:(qkc + 1) * P, :])
                for n in range(2):
                    nc.tensor.matmul(
                        proj_ps["q"][n][:],
                        lhsT=wq_sb[:, qkc, :],
                        rhs=xq[:, n * 512:(n + 1) * 512],
                        start=(qkc == 0), stop=(qkc == KC - 1),
                    )
                if qkc == KC - 1:
                    for n in range(2):
                        nc.any.tensor_copy(
                            qT_sb[:, n * 512:(n + 1) * 512],
                            proj_ps["q"][n][:])

            def kT_chunklet(chalf, kkc, ps_pool=None, ps_tag="pj"):
                if kkc == 0:
                    if chalf == 0:
                        nc.sync.dma_start(
                            wk_sb[:],
                            wkT.rearrange("(kc p) m -> p kc m", p=P))
                    pool_ = ps_pool if ps_pool is not None else pj
                    proj_ps[chalf] = [
                        pool_.tile([P, 512], f32, tag=f"{ps_tag}{n}",
                                   name=f"ps_k{chalf}_{n}") for n in range(2)]
                ps_k = proj_ps[chalf]
                ct = stream.tile([P, NQ], f32r, tag="ct",
                                 name=f"ct{chalf}_{kkc}", bufs=3)
                nc.sync.dma_start(
                    ct[:], cT[kkc * P:(kkc + 1) * P,
                              chalf * NQ:(chalf + 1) * NQ])
                for n in range(2):
                    nc.tensor.matmul(
                        ps_k[n][:],
                        lhsT=wk_sb[:, kkc, :],
                        rhs=ct[:, n * 512:(n + 1) * 512],
                        start=(kkc == 0), stop=(kkc == KC - 1),
                    )
                if kkc == KC - 1:
                    for n in range(2):
                        nc.any.tensor_copy(
                            kT_sb[:, chalf * NQ + n * 512:
                                  chalf * NQ + (n + 1) * 512], ps_k[n][:])

            psv = phase_ctx.enter_context(
                tc.tile_pool(name="psv", bufs=1, space="PSUM"))
            if True:

                xT_r = xT.rearrange("(kc p) s -> p kc s", p=P)

                def v_eighth(se, dma_hook=None, mm_hook=None,
                             defer_copies=False, split_xt=False):
                    # One strided DMA loads all 8 kc chunks of this eighth
                    # (each dma_start costs ~650ns of SP sequencer issue
                    # time, so fewer/bigger DMAs beat per-chunk loads).
                    # split_xt: ve0 loads per-kc so the PE can consume each
                    # (wv[kc], xt[kc]) pair as it lands at startup.
                    if split_xt:
                        xta = xpool.tile([P, KC, 256], f32r, tag="xts",
                                         name=f"xts{se}", bufs=2)
                        for kc in range(KC):
                            if dma_hook is not None:
                                dma_hook(kc)
                            nc.sync.dma_start(
                                xta[:, kc, :],
                                xT[kc * P:(kc + 1) * P,
                                   se * 256:(se + 1) * 256])
                    else:
                        xta = xpool.tile([P, KC, 256], f32r, tag="xt",
                                         name=f"xta{se}")
                        # kc-quarter loads: the kc-ascending matmuls
                        # start as soon as the first quarter lands
                        for h in range(4):
                            nc.sync.dma_start(
                                xta[:, 2 * h:2 * h + 2, :],
                                xT_r[:, 2 * h:2 * h + 2,
                                     se * 256:(se + 1) * 256])
                        if dma_hook is not None:
                            for kc in range(KC):
                                dma_hook(kc)
                    pss = [psv.tile([P, 512], f32, tag=f"psv{i}",
                                    name=f"psv{se}_{i}") for i in range(4)]
                    for kc in range(KC):
                        for n in range(2):
                            for sb in range(2):
                                nc.tensor.matmul(
                                    pss[sb * 2 + n][:],
                                    lhsT=xta[:, kc, sb * P:(sb + 1) * P],
                                    rhs=wv_sb[:, kc, n * 512:(n + 1) * 512],
                                    start=(kc == 0), stop=(kc == KC - 1),
                                )
                        if mm_hook is not None:
                            mm_hook(kc)
                    def flush():
                        for sb in range(2):
                            for n in range(2):
                                nc.any.tensor_copy(
                                    v_sb[:, se * 2 + sb,
                                         n * 512:(n + 1) * 512],
                                    pss[sb * 2 + n][:])
                    if defer_copies:
                        return flush
                    flush()
                    return None

                v_eighth(0, dma_hook=lambda kc: nc.sync.dma_start(
                    wv_sb[:, kc, :], wvT_r[kc]), split_xt=True)
                # qT / kT-chalf0 chunklets spread 2-3 per eighth so each
                # hosting eighth stays DMA-surplus-positive.
                def qh(base):
                    return lambda kc: qT_chunklet(base + kc // 4) \
                        if kc % 4 == 3 else None

                def kh(lst):
                    return lambda kc: kT_chunklet(0, lst[kc // 3]) \
                        if kc % 3 == 1 and kc // 3 < len(lst) else None

                v_eighth(1, mm_hook=qh(0), split_xt=True)
                v_eighth(2, mm_hook=qh(2))
                v_eighth(3, mm_hook=qh(4))
                v_eighth(4, mm_hook=qh(6))
                v_eighth(5, mm_hook=kh([0, 1, 2]))
                v_eighth(6, mm_hook=kh([3, 4, 5]))
                # the SP FIFO tail is xt-slot gated, so these mask loads
                # slip into idle DMA windows without delaying ve7
                nc.sync.dma_start(qmn_sb[:], qmn[:])
                nc.sync.dma_start(dm_sb[:], dmask.rearrange("s p t -> p s t"))
                nc.sync.dma_start(kmb_sb[:], kmb[:])
                v_eighth(7, mm_hook=kh([6, 7]))

        # ---- attention (stage A: logits->exp->transpose; B: AV) ----
        if True:
            apool = ctx.enter_context(tc.tile_pool(name="apool", bufs=5))
            epool = ctx.enter_context(tc.tile_pool(name="epool", bufs=24))
            with tc.tile_pool(name="psl", bufs=2, space="PSUM") as psl_pool, \
                 tc.tile_pool(name="psT", bufs=2, space="PSUM") as psT_pool, \
                 tc.tile_pool(name="psav", bufs=2, space="PSUM") as psav_pool, \
                 tc.tile_pool(name="pjk", bufs=1, space="PSUM") as pjk_pool:

                def stage_a_chunk(pr, j, mid_hook=None):
                    c = PAIR_C[pr]
                    psTs = [psT_pool.tile([P, 256], f32r, tag="psT",
                                          name=f"psT{pr}_{j}_{ks}",
                                          padded_shape=[P, 512])
                            for ks in range(4)]
                    es = []
                    for blk in range(2):
                        slot = pr * 2 + blk
                        psl = psl_pool.tile([P, 512], f32, tag="psl",
                                            name=f"psl{slot}_{j}")
                        nc.tensor.matmul(
                            psl[:],
                            lhsT=qT_sb[:, slot * P:(slot + 1) * P],
                            rhs=kT_sb[:, j * 512:(j + 1) * 512],
                            start=True, stop=True,
                        )
                        sbl = apool.tile([P, 512], f32, tag="sbl",
                                         name=f"sbl{slot}_{j}")
                        add_src = dm_sb[:, slot, :] if j == c - 1 \
                            else kmb_sb[:, j * 512:(j + 1) * 512]
                        nc.vector.tensor_tensor(
                            out=sbl[:], in0=psl[:], in1=add_src, op=ALU.add)
                        e = apool.tile([P, 512], f32r, tag="e",
                                       name=f"e{slot}_{j}")
                        dac = apool.tile([P, 1], f32, tag="dac",
                                         name=f"dac{slot}_{j}", bufs=10)
                        nc.scalar.activation(
                            e[:], sbl[:], AF.Exp,
                            bias=qmn_sb[:, slot:slot + 1], scale=1.0,
                            accum_out=dac[:],
                        )
                        daccs_all[pr][blk].append(dac)
                        es.append(e)
                    # PE work emitted here hides the DVE-add + exp latency
                    # before the transposes need the exp outputs
                    if mid_hook is not None:
                        mid_hook()
                    for blk in range(2):
                        for ks in range(4):
                            nc.tensor.transpose(
                                psTs[ks][:, blk * P:(blk + 1) * P],
                                es[blk][:, ks * P:(ks + 1) * P],
                                ident[:],
                            )
                    for ks in range(4):
                        eT = epool.tile([P, 256], f32r, tag="eT",
                                        name=f"eT{pr}_{j}_{ks}")
                        nc.any.tensor_copy(eT[:], psTs[ks][:])
                        eTs_all[pr].append(eT)

                def stage_b(pr, use_pjk=False, den_dma=False):
                    c = PAIR_C[pr]
                    eTs = eTs_all[pr]
                    for blk in range(2):
                        slot = pr * 2 + blk
                        dl = daccs_all[pr][blk]
                        dst = den_sb[:, slot:slot + 1]
                        if c == 1:
                            nc.any.tensor_copy(dst, dl[0][:])
                        else:
                            nc.vector.tensor_tensor(
                                out=dst, in0=dl[0][:], in1=dl[1][:],
                                op=ALU.add)
                            for d in dl[2:]:
                                nc.vector.tensor_tensor(
                                    out=dst, in0=dst, in1=d[:], op=ALU.add)
                    if den_dma:
                        # all 8 den slots are final here; flush during the AV
                        nc.sync.dma_start(den[:], den_sb[:])
                    for dmc in range(8):
                        if use_pjk and dmc % 4 >= 2:
                            # kT-chalf1's pjk banks are dead by R3: reuse
                            # them as two extra AV slots (depth 4 pipeline)
                            psav = pjk_pool.tile(
                                [P, 512], f32, tag=f"pjk{dmc % 2}",
                                name=f"psav{pr}_{dmc}")[:, :256]
                        else:
                            psav = psav_pool.tile([P, 256], f32, tag="psav",
                                                  name=f"psav{pr}_{dmc}")
                        for kb in range(4 * c):
                            nc.tensor.matmul(
                                psav[:],
                                lhsT=v_sb[:, kb, dmc * P:(dmc + 1) * P],
                                rhs=eTs[kb][:],
                                start=(kb == 0), stop=(kb == 4 * c - 1),
                            )
                        osb = apool.tile([P, 256], f32, tag="osb",
                                         name=f"osb{pr}_{dmc}")
                        nc.any.tensor_copy(osb[:], psav[:])
                        nc.sync.dma_start(
                            outT[dmc * P:(dmc + 1) * P,
                                 pr * 256:(pr + 1) * 256], osb[:])

                # R2: pairs 0 and 2 interleaved with kT chalf1 (psum on psT tag)
                def kt1_first():
                    for kkc in range(4):
                        kT_chunklet(1, kkc, ps_pool=pjk_pool, ps_tag="pjk")

                def kt1_second():
                    for kkc in range(4, 8):
                        kT_chunklet(1, kkc, ps_pool=pjk_pool, ps_tag="pjk")

                stage_a_chunk(0, 0, mid_hook=kt1_first)
                stage_b(0)
                stage_a_chunk(2, 0, mid_hook=kt1_second)
                stage_a_chunk(2, 1)
                stage_b(2)
                # R3: pair 1 then 3, A3 interleaved before B1
                for j in range(4):
                    stage_a_chunk(1, j)
                stage_a_chunk(3, 0)
                stage_b(1, use_pjk=True)
                stage_a_chunk(3, 1)
                stage_a_chunk(3, 2)
                stage_b(3, use_pjk=True, den_dma=True)


    nc.compile()
    return nc


def _get_nc():
    with _BUILD_LOCK:
        if "nc" not in _CACHE:
            _CACHE["nc"] = _build()
        return _CACHE["nc"]


def kernel(x, cross, Wq, Wk, Wv, mask):
    from concourse import bass_utils

    nc = _get_nc()

    x = np.asarray(x, dtype=np.float32)
    cross = np.asarray(cross, dtype=np.float32)
    scale = 1.0 / math.sqrt(DA)
    wqT_h = np.ascontiguousarray((np.asarray(Wq, np.float32) * scale).T)
    wkT_h = np.ascontiguousarray(np.asarray(Wk, np.float32).T)
    wvT_h = np.ascontiguousarray(np.asarray(Wv, np.float32).T)
    mf = np.asarray(mask).astype(np.float32)  # [B, S]

    karange = np.arange(S)
    in_maps = []
    rows_per_core = []
    for core in range(NCORES):
        b, p = divmod(core, 2)
        blocks = STRIPS[p]
        rows = np.concatenate([np.arange(g * P, (g + 1) * P) for g in blocks])
        rows_per_core.append((b, rows))
        mb = mf[b]
        kneg = (-BIG * (1.0 - mb)).astype(np.float32)  # [S]
        kmb_h = np.ascontiguousarray(
            np.broadcast_to(kneg[:1536], (P, 1536))).astype(ml_dtypes.bfloat16)
        mq = mb[rows]  # [1024]
        qmn_h = np.ascontiguousarray(
            (-BIG * (1.0 - mq)).reshape(8, P).T)  # [128, 8]
        dm_h = np.empty((8, P, 512), np.float32)
        for s, g in enumerate(blocks):
            c = PAIR_C[s // 2]
            k0 = (c - 1) * 512
            kk = karange[k0:k0 + 512]
            qq = g * P + np.arange(P)
            mqs = mq[s * P:(s + 1) * P]
            t = np.broadcast_to(kneg[k0:k0 + 512], (P, 512)).copy()
            t += -BIG * (kk[None, :] > qq[:, None])
            t += (2.0 * BIG * (1.0 - mqs))[:, None] * (kk[None, :] == qq[:, None])
            dm_h[s] = t
        in_maps.append({
            "xT": np.ascontiguousarray(x[b].T),
            "cT": np.ascontiguousarray(cross[b].T),
            "xqT": np.ascontiguousarray(x[b][rows].T),
            "wqT": wqT_h,
            "wkT": wkT_h,
            "wvT": wvT_h,
            "kmb": kmb_h,
            "qmn": qmn_h,
            "dmask": dm_h.astype(ml_dtypes.bfloat16),
        })

    _CACHE["in_maps"] = in_maps
    res = bass_utils.run_bass_kernel_spmd(
        nc, in_maps, core_ids=list(range(NCORES)))

    out = np.empty((B, S, D), np.float32)
    for core in range(NCORES):
        b, rows = rows_per_core[core]
        r = res.results[core]
        o = r["outT"].T  # [1024 q, 1024 dm]
        denf = r["den"].T.reshape(-1)  # [1024] strip-ordered
        out[b, rows] = o / denf[:, None]
    return out
